# revision 7
# baseline (speedup 1.0000x reference)
"""GIN message-passing GNN (4 layers + BN + graph pooling) on 8 Trainium2 cores.

Strategy (dst-sharded node partition):
  - Nodes (and their incoming edges) are partitioned contiguously across 8 cores.
  - Per layer, each core aggregates h[src] over its edges with dma_gather
    (custom gather instruction; int16 indices, so the replicated bf16 node
    table is addressed in 25000-row banks) followed by per-128-edge-block
    selection matmuls accumulated in PSUM (exact segment-sum incl. duplicate
    destinations).
  - The GIN self term (1+eps)*h is one extra matmul per tile against a scaled
    identity, reading the core's own previous-layer shard at static addresses.
  - The 2-layer MLP runs in a transposed layout (features on partitions) so no
    transposes are needed; weights are the stationary matmul operand.
  - BatchNorm statistics are summed per-core then all-reduced (1KB collective).
  - Post-BN node features are written to a bf16 shard and exchanged with an
    AllGather so the next layer can gather from the full table.
  - Graph pooling (segment-sum over sorted `batch`) runs per core with
    selection matmuls into a persistent PSUM accumulator; the final linear runs
    on-device; the host sums per-core partial outputs (boundary graphs get
    contributions from two cores) and adds bout.
"""

import math

import numpy as np
import ml_dtypes

P = 128  # partitions / tile edge

FULL_CFG = dict(N=100000, F=128, H=128, L=4, G=512, O=10, C=8, GT=5)


def _cdiv(a, b):
    return (a + b - 1) // b


def prep_host_data(x, edge_index, batch, cfg):
    """Partition edges by destination core/tile and source bank; build padded
    per-core gather-index (int16, dma_gather packing) and selection arrays."""
    N, F, G, C = cfg["N"], cfg["F"], cfg["G"], cfg["C"]
    NP = N // C
    T = _cdiv(NP, P)
    bank_rows = cfg.get("bank_rows") or NP * max(1, min(C, 32767 // NP))
    NBANK = _cdiv(N, bank_rows)
    src = np.asarray(edge_index[0], dtype=np.int64)
    dst = np.asarray(edge_index[1], dtype=np.int64)
    batch = np.asarray(batch, dtype=np.int64)

    order = np.argsort(dst, kind="stable")
    dsts = dst[order]
    srcs = src[order]
    core_of = dsts // NP
    loc = dsts - core_of * NP
    lt = loc // P
    dloc_all = loc - lt * P
    bank_all = srcs // bank_rows

    # counts per (core, tile, bank)
    key = (core_of * T + lt) * NBANK + bank_all
    cnt = np.bincount(key, minlength=C * T * NBANK).reshape(C, T, NBANK)
    Bk = _cdiv(cnt, P).max(axis=0)  # [T, NBANK] blocks, shared across cores
    nblk = Bk.sum(axis=1)  # gathered blocks per tile
    blk_off = np.concatenate([[0], np.cumsum(nblk)])[:-1]
    Btot = int(nblk.sum())
    nblkmax = int(nblk.max())

    # per (t,bank): block offset within tile, idx-row offset globally
    bank_blk_off = np.zeros((T, NBANK), dtype=np.int64)
    for t in range(T):
        bank_blk_off[t] = np.concatenate([[0], np.cumsum(Bk[t])])[:-1]
    rows_per = Bk * P  # gather rows per (t,bank)
    roff = np.zeros((T, NBANK), dtype=np.int64)
    acc = 0
    for t in range(T):
        for b in range(NBANK):
            roff[t, b] = acc
            acc += rows_per[t, b]
    tot_rows = int(acc)

    idx16 = np.zeros((C, 16, tot_rows // 16), dtype=np.int16)
    dst_loc = np.full((C, P, Btot), 200.0, dtype=ml_dtypes.bfloat16)

    # order edges within (core,tile) by bank; fill arrays
    order2 = np.lexsort((bank_all, lt, core_of))
    srcs2, dloc2 = srcs[order2], dloc_all[order2]
    cnt_flat = cnt.reshape(-1)
    starts = np.zeros(C * T * NBANK + 1, dtype=np.int64)
    np.cumsum(cnt_flat, out=starts[1:])
    for c in range(C):
        for t in range(T):
            for b in range(NBANK):
                kb = int(Bk[t, b])
                if kb == 0:
                    continue
                i0 = starts[(c * T + t) * NBANK + b]
                i1 = starts[(c * T + t) * NBANK + b + 1]
                n = int(i1 - i0)
                K = kb * P
                li = np.zeros(K, dtype=np.int16)
                li[:n] = (srcs2[i0:i1] - b * bank_rows).astype(np.int16)
                r0 = roff[t, b]
                idx16[c, :, r0 // 16 : (r0 + K) // 16] = li.reshape(K // 16, 16).T
                dl = np.full(K, 200.0)
                dl[:n] = dloc2[i0:i1]
                co = blk_off[t] + bank_blk_off[t, b]
                dst_loc[c, :, co : co + kb] = (
                    dl.reshape(kb, P).T.astype(ml_dtypes.bfloat16)
                )

    # batch ids, local per core: [C, P, T]
    gbase = np.array([batch[c * NP] for c in range(C)], dtype=np.int64)
    batch_loc = np.full((C, P, T), 200.0, dtype=ml_dtypes.bfloat16)
    for c in range(C):
        bl = batch[c * NP : (c + 1) * NP] - gbase[c]
        assert bl.max() < P, f"graph span {bl.max() + 1} exceeds {P} on core {c}"
        pad = np.full(T * P - NP, 200.0)
        batch_loc[c] = (
            np.concatenate([bl.astype(np.float64), pad])
            .reshape(T, P)
            .T.astype(ml_dtypes.bfloat16)
        )

    iota_rep = np.tile(
        np.arange(P, dtype=np.float64), (P, max(nblkmax, 1))
    ).astype(ml_dtypes.bfloat16)
    ident = np.eye(P, dtype=np.float32)
    identbf = np.eye(P).astype(ml_dtypes.bfloat16)

    return dict(
        NP=NP, T=T, NBANK=NBANK, bank_rows=bank_rows, Bk=Bk, nblk=nblk,
        blk_off=blk_off, bank_blk_off=bank_blk_off, roff=roff, rows_per=rows_per,
        tot_rows=tot_rows, Btot=Btot, nblkmax=nblkmax, idx16=idx16,
        dst_loc=dst_loc, batch_loc=batch_loc, gbase=gbase, iota_rep=iota_rep,
        ident=ident, identbf=identbf,
    )


def build_program(cfg, hd, eps_host, trn_type="TRN2"):
    """Build the Bass/Tile program (single SPMD program, per-core data differs)."""
    import concourse.tile as tile
    from concourse import bacc, mybir

    N, F, H, L, G, O, C = (cfg[k] for k in ("N", "F", "H", "L", "G", "O", "C"))
    GT = cfg.get("GT", 5)  # tiles per dma_gather group
    NP, T, NBANK, bank_rows = hd["NP"], hd["T"], hd["NBANK"], hd["bank_rows"]
    Bk, nblk, blk_off, bank_blk_off = hd["Bk"], hd["nblk"], hd["blk_off"], hd["bank_blk_off"]
    roff, rows_per, tot_rows, Btot, nblkmax = (
        hd["roff"], hd["rows_per"], hd["tot_rows"], hd["Btot"], hd["nblkmax"]
    )
    f32 = mybir.dt.float32
    bf16 = mybir.dt.bfloat16
    i16 = mybir.dt.int16
    AF = mybir.ActivationFunctionType
    ALU = mybir.AluOpType

    nc = bacc.Bacc(trn_type, target_bir_lowering=False, debug=False, num_devices=C)

    # ---- external inputs -------------------------------------------------
    x_table_e = nc.dram_tensor("x_table", [N, F], bf16, kind="ExternalInput")
    x_own_e = nc.dram_tensor("x_own", [T * P, F], bf16, kind="ExternalInput")
    idx16_e = nc.dram_tensor("idx16", [P, _cdiv(tot_rows, 16)], i16, kind="ExternalInput")
    dst_loc_e = nc.dram_tensor("dst_loc", [P, Btot], bf16, kind="ExternalInput")
    batch_loc_e = nc.dram_tensor("batch_loc", [P, T], bf16, kind="ExternalInput")
    iota_rep_e = nc.dram_tensor("iota_rep", [P, max(nblkmax, 1) * P], bf16, kind="ExternalInput")
    ident_e = nc.dram_tensor("ident", [P, P], f32, kind="ExternalInput")
    identbf_e = nc.dram_tensor("identbf", [P, P], bf16, kind="ExternalInput")
    w1_e = nc.dram_tensor("W1s", [L, F, H], f32, kind="ExternalInput")
    w2_e = nc.dram_tensor("W2s", [L, H, H], f32, kind="ExternalInput")
    b1t_e = nc.dram_tensor("b1T", [H, L], f32, kind="ExternalInput")
    b2t_e = nc.dram_tensor("b2T", [H, L], f32, kind="ExternalInput")
    gamt_e = nc.dram_tensor("gamT", [H, L], f32, kind="ExternalInput")
    bett_e = nc.dram_tensor("betT", [H, L], f32, kind="ExternalInput")
    wout_e = nc.dram_tensor("Wout", [(F + L * H), O], f32, kind="ExternalInput")
    out_e = nc.dram_tensor("outT", [O, P], f32, kind="ExternalOutput")

    NCHUNK = 1 + L
    stage = cfg.get("stage", "F")
    # tile-groups for gathers
    groups_t = [list(range(g, min(g + GT, T))) for g in range(0, T, GT)]

    with tile.TileContext(nc) as tc:
        import contextlib

        with contextlib.ExitStack() as ctx:
            const = ctx.enter_context(tc.tile_pool(name="const", bufs=1))
            dram = ctx.enter_context(tc.tile_pool(name="dram", bufs=1, space="DRAM"))
            gpool = ctx.enter_context(tc.tile_pool(name="gpool", bufs=2))
            spool = ctx.enter_context(tc.tile_pool(name="spool", bufs=3))
            zpool = ctx.enter_context(tc.tile_pool(name="zpool", bufs=2))
            hpool = ctx.enter_context(tc.tile_pool(name="hpool", bufs=3))
            stpool = ctx.enter_context(tc.tile_pool(name="stpool", bufs=2))
            smpool = ctx.enter_context(tc.tile_pool(name="smpool", bufs=2))
            wpool = ctx.enter_context(tc.tile_pool(name="wpool", bufs=2))
            psA = ctx.enter_context(tc.tile_pool(name="psA", bufs=2, space="PSUM"))
            psB = ctx.enter_context(tc.tile_pool(name="psB", bufs=1, space="PSUM"))
            psC = ctx.enter_context(tc.tile_pool(name="psC", bufs=1, space="PSUM"))
            psT = ctx.enter_context(tc.tile_pool(name="psT", bufs=2, space="PSUM"))
            psP = ctx.enter_context(tc.tile_pool(name="psP", bufs=1, space="PSUM"))

            # ---- constants loaded once ---------------------------------
            idx_sb = const.tile([P, _cdiv(tot_rows, 16)], i16)
            nc.sync.dma_start(out=idx_sb[:], in_=idx16_e[:, :])
            dloc_sb = const.tile([P, Btot], bf16)
            nc.sync.dma_start(out=dloc_sb[:], in_=dst_loc_e[:, :])
            bloc_sb = const.tile([P, T], bf16)
            nc.sync.dma_start(out=bloc_sb[:], in_=batch_loc_e[:, :])
            iota_sb = const.tile([P, max(nblkmax, 1) * P], bf16)
            nc.sync.dma_start(out=iota_sb[:], in_=iota_rep_e[:, :])
            ident_sb = const.tile([P, P], f32)
            nc.sync.dma_start(out=ident_sb[:], in_=ident_e[:, :])
            identbf_sb = const.tile([P, P], bf16)
            nc.sync.dma_start(out=identbf_sb[:], in_=identbf_e[:, :])
            b1_sb = const.tile([P, L], f32)
            nc.sync.dma_start(out=b1_sb[:], in_=b1t_e[:, :])
            b2_sb = const.tile([P, L], f32)
            nc.sync.dma_start(out=b2_sb[:], in_=b2t_e[:, :])
            gam_sb = const.tile([P, L], f32)
            nc.sync.dma_start(out=gam_sb[:], in_=gamt_e[:, :])
            bet_sb = const.tile([P, L], f32)
            nc.sync.dma_start(out=bet_sb[:], in_=bett_e[:, :])

            pooled_sb = [const.tile([P, P], f32, name=f"pooled{i}") for i in range(NCHUNK)]

            # DRAM internals
            ag_bufs = [dram.tile([T * P, F], bf16, name=f"ag{i}") for i in range(2)]
            n_tables = max(0, min(2, L - 1))
            tables = [dram.tile([N, F], bf16, name=f"table{i}") for i in range(n_tables)]
            stats_in = dram.tile([P, 2], f32, name="stats_in")
            stats_out = dram.tile([P, 2], f32, name="stats_out")

            cgroups = [list(range(C))]

            # zero the padded tail rows of ag bufs (avoid NaN junk in self term)
            if T * P > NP:
                ztail = const.tile([P, F], bf16, name="ztail")
                nc.vector.memset(ztail[:], 0)
                for agb in ag_bufs:
                    nc.sync.dma_start(out=agb[NP : T * P, :], in_=ztail[: T * P - NP, :])

            # ---- pooled chunk 0: x ------------------------------------
            pooledp = psP.tile([P, P], f32, name="pooledp_x", tag="pooledp")
            for t in range(T):
                xt = hpool.tile([P, P], bf16, name="xt", tag="selft")
                nc.sync.dma_start(out=xt[:], in_=x_own_e[t * P : (t + 1) * P, :])
                selb = spool.tile([P, P], bf16, name="selb_x", tag="selb")
                nc.vector.tensor_tensor(
                    out=selb[:],
                    in0=bloc_sb[:, t : t + 1].to_broadcast([P, P]),
                    in1=iota_sb[:, :P],
                    op=ALU.is_equal,
                )
                nc.tensor.matmul(
                    pooledp[:], lhsT=xt[:], rhs=selb[:],
                    start=(t == 0), stop=(t == T - 1),
                )
            nc.scalar.copy(pooled_sb[0][:], pooledp[:])

            # ---- layers ------------------------------------------------
            z2all = const.tile([P, T * P], f32, name="z2all")
            LB = 0 if stage == "A" else (1 if stage in ("B", "C", "D", "E") else L)
            for l in range(LB):
                table_ap = x_table_e.ap() if l == 0 else tables[(l - 1) % 2][:]
                own_prev = x_own_e.ap() if l == 0 else ag_bufs[(l - 1) % 2][:]
                w1t = wpool.tile([P, H], f32, name="w1t", tag="w1")
                nc.sync.dma_start(out=w1t[:], in_=w1_e[l, :, :])
                w2t = wpool.tile([P, H], f32, name="w2t", tag="w2")
                nc.sync.dma_start(out=w2t[:], in_=w2_e[l, :, :])

                one_p_eps = 1.0 + float(eps_host[l])
                if one_p_eps != 1.0:
                    ideps = wpool.tile([P, P], bf16, name="ideps", tag="ideps")
                    nc.vector.tensor_scalar_mul(ideps[:], identbf_sb[:], one_p_eps)
                else:
                    ideps = identbf_sb

                sums = stpool.tile([P, T], f32, name="sums", tag="sums")
                sqs = stpool.tile([P, T], f32, name="sqs", tag="sqs")

                # gathers per tile-group x bank into one big buffer per group
                gbufs = {}
                for tg in groups_t:
                    t0 = tg[0]
                    gb_blocks = int(nblk[tg[0] : tg[-1] + 1].sum())
                    gbuf = gpool.tile(
                        [P, GT * nblkmax, P], bf16, name="gbuf", tag="gbuf"
                    )
                    gbufs[t0] = (gbuf, int(blk_off[t0]))
                    cur = 0
                    for b in range(NBANK):
                        for t in tg:
                            kb = int(Bk[t, b])
                            if kb == 0:
                                continue
                            K = kb * P
                            r0 = int(roff[t, b])
                            bo = int(blk_off[t] + bank_blk_off[t, b] - blk_off[t0])
                            nc.gpsimd.dma_gather(
                                out_ap=gbuf[:, bo : bo + kb, :],
                                in_ap=table_ap[
                                    b * bank_rows : min((b + 1) * bank_rows, N), :
                                ],
                                idxs_ap=idx_sb[:, r0 // 16 : (r0 + K) // 16],
                                num_idxs=K,
                                num_idxs_reg=K,
                                elem_size=F,
                            )

                sub = cfg.get("sub", 9)
                for t in range(T):
                    nb = int(nblk[t])
                    co = int(blk_off[t])
                    vd = min(P, NP - t * P)
                    gbuf, gb_off = gbufs[(t // GT) * GT]
                    gsl = gbuf[:, co - gb_off : co - gb_off + nb, :]
                    if sub == 1:
                        junk1 = zpool.tile([P, P], f32, name="junk1", tag="junk")
                        nc.vector.tensor_copy(out=junk1[:], in_=gsl[:, 0, :])
                        nc.sync.dma_start(out=out_e[:, :], in_=junk1[:O, :])
                        continue
                    sel = spool.tile([P, nblkmax, P], bf16, name="sel", tag="sel")
                    nc.vector.tensor_tensor(
                        out=sel[:, :nb, :],
                        in0=dloc_sb[:, co : co + nb].to_broadcast([P, nb, P]),
                        in1=iota_sb[:, : nb * P].rearrange("p (b e) -> p b e", b=nb),
                        op=ALU.is_equal,
                    )
                    if sub == 2:
                        junk1 = zpool.tile([P, P], f32, name="junk1", tag="junk")
                        nc.vector.tensor_copy(out=junk1[:], in_=sel[:, 0, :])
                        nc.sync.dma_start(out=out_e[:, :], in_=junk1[:O, :])
                        continue
                    selft = hpool.tile([P, P], bf16, name="selft", tag="selft")
                    nc.sync.dma_start(
                        out=selft[:], in_=own_prev[t * P : (t + 1) * P, :]
                    )
                    z0p = psA.tile([P, P], f32, name="z0p", tag="z0p")
                    nc.tensor.matmul(
                        z0p[:], lhsT=selft[:], rhs=ideps[:], start=True, stop=False
                    )
                    for b in range(nb):
                        nc.tensor.matmul(
                            z0p[:], lhsT=gsl[:, b, :], rhs=sel[:, b, :],
                            start=False, stop=(b == nb - 1),
                        )
                    z0t = zpool.tile([P, P], f32, name="z0t", tag="z0t")
                    nc.scalar.copy(z0t[:], z0p[:])
                    if sub == 3:
                        nc.sync.dma_start(out=out_e[:, :], in_=z0t[:O, :])
                        continue
                    z1p = psB.tile([P, P], f32, name="z1p", tag="z1p")
                    nc.tensor.matmul(z1p[:], lhsT=w1t[:], rhs=z0t[:], start=True, stop=True)
                    z1t = zpool.tile([P, P], f32, name="z1t", tag="z1t")
                    nc.scalar.activation(z1t[:], z1p[:], AF.Relu, bias=b1_sb[:, l : l + 1])
                    z2p = psC.tile([P, P], f32, name="z2p", tag="z2p")
                    nc.tensor.matmul(z2p[:], lhsT=w2t[:], rhs=z1t[:], start=True, stop=True)
                    z2t = z2all[:, t * P : (t + 1) * P]
                    nc.scalar.activation(z2t, z2p[:], AF.Identity, bias=b2_sb[:, l : l + 1])
                    if sub == 4:
                        nc.sync.dma_start(out=out_e[:, :], in_=z2all[:O, t * P : (t + 1) * P])
                        continue
                    nc.vector.tensor_reduce(
                        sums[:, t : t + 1], z2t[:, :vd], axis=mybir.AxisListType.X, op=ALU.add
                    )
                    if sub == 5:
                        nc.sync.dma_start(out=out_e[:, :], in_=z2all[:O, t * P : (t + 1) * P])
                        continue
                    junk = zpool.tile([P, P], f32, name="junk", tag="junk")
                    nc.scalar.activation(
                        junk[:, :vd], z2t[:, :vd], AF.Square,
                        accum_out=sqs[:, t : t + 1],
                    )
                    if sub == 6:
                        nc.sync.dma_start(out=out_e[:, :], in_=z2all[:O, t * P : (t + 1) * P])
                        continue

                # ---- BN stats all-reduce -------------------------------
                if sub < 9:
                    break
                pack = smpool.tile([P, 2], f32, name="pack", tag="pack")
                nc.vector.tensor_reduce(
                    pack[:, 0:1], sums[:, :T], axis=mybir.AxisListType.X, op=ALU.add
                )
                nc.vector.tensor_reduce(
                    pack[:, 1:2], sqs[:, :T], axis=mybir.AxisListType.X, op=ALU.add
                )
                nc.sync.dma_start(out=stats_in[:], in_=pack[:])
                if stage != "B":
                    nc.gpsimd.collective_compute(
                        "AllReduce", ALU.add, replica_groups=cgroups,
                        ins=[stats_in.opt()], outs=[stats_out.opt()],
                    )
                statg = smpool.tile([P, 2], f32, name="statg", tag="statg")
                nc.sync.dma_start(out=statg[:], in_=(stats_in if stage == "B" else stats_out)[:])
                if stage in ("B", "C"):
                    continue
                mu = smpool.tile([P, 1], f32, name="mu", tag="mu")
                nc.scalar.mul(mu[:], statg[:, 0:1], 1.0 / N)
                ex2 = smpool.tile([P, 1], f32, name="ex2", tag="ex2")
                nc.scalar.mul(ex2[:], statg[:, 1:2], 1.0 / N)
                var = smpool.tile([P, 1], f32, name="var", tag="var")
                nc.vector.tensor_tensor(out=var[:], in0=mu[:], in1=mu[:], op=ALU.mult)
                nc.vector.tensor_tensor(out=var[:], in0=ex2[:], in1=var[:], op=ALU.subtract)
                nc.vector.tensor_scalar_add(var[:], var[:], 1e-5)
                vst = smpool.tile([P, 1], f32, name="vst", tag="vst")
                nc.scalar.activation(vst[:], var[:], AF.Sqrt)
                rstd = smpool.tile([P, 1], f32, name="rstd", tag="rstd")
                nc.vector.reciprocal(rstd[:], vst[:])
                acol = smpool.tile([P, 1], f32, name="acol", tag="acol")
                nc.vector.tensor_tensor(out=acol[:], in0=gam_sb[:, l : l + 1], in1=rstd[:], op=ALU.mult)
                bcol = smpool.tile([P, 1], f32, name="bcol", tag="bcol")
                nc.vector.tensor_tensor(out=bcol[:], in0=mu[:], in1=acol[:], op=ALU.mult)
                nc.vector.tensor_tensor(out=bcol[:], in0=bet_sb[:, l : l + 1], in1=bcol[:], op=ALU.subtract)

                # ---- BN apply + pooled + table shard -------------------
                has_next = (l < L - 1) and stage not in ("D",)
                pooledp = psP.tile([P, P], f32, name=f"pooledp{l}", tag="pooledp")
                for t in range(T):
                    vd = min(P, NP - t * P)
                    z2t = z2all[:, t * P : (t + 1) * P]
                    hT = zpool.tile([P, P], f32, name="hT", tag="hT")
                    nc.scalar.activation(hT[:], z2t, AF.Relu, bias=bcol[:, 0:1], scale=acol[:, 0:1])
                    trp = psT.tile([P, P], f32, name="trp", tag="trp")
                    nc.tensor.transpose(trp[:], hT[:], ident_sb[:])
                    hbf = hpool.tile([P, P], bf16, name="hbf", tag="hbf")
                    nc.vector.tensor_copy(out=hbf[:], in_=trp[:])
                    if has_next:
                        nc.sync.dma_start(
                            out=ag_bufs[l % 2][t * P : t * P + vd, :], in_=hbf[:vd, :]
                        )
                    selb = spool.tile([P, P], bf16, name="selb", tag="selb")
                    nc.vector.tensor_tensor(
                        out=selb[:],
                        in0=bloc_sb[:, t : t + 1].to_broadcast([P, P]),
                        in1=iota_sb[:, :P],
                        op=ALU.is_equal,
                    )
                    nc.tensor.matmul(
                        pooledp[:], lhsT=hbf[:], rhs=selb[:],
                        start=(t == 0), stop=(t == T - 1),
                    )
                nc.scalar.copy(pooled_sb[1 + l][:], pooledp[:])

                if has_next:
                    nc.gpsimd.collective_compute(
                        "AllGather", ALU.bypass, replica_groups=cgroups,
                        ins=[ag_bufs[l % 2][:NP, :].opt()], outs=[tables[l % 2].opt()],
                    )

            # ---- output head ------------------------------------------
            with tc.tile_pool(name="psO", bufs=1, space="PSUM") as psO:
                outp = psO.tile([O, P], f32, name="outp")
                nchunk_built = 1 + (LB if stage not in ("B", "C") else 0)
                for i in range(nchunk_built):
                    wc = wpool.tile([P, O], f32, name="wc", tag="wc")
                    nc.sync.dma_start(out=wc[:], in_=wout_e[i * P : (i + 1) * P, :])
                    nc.tensor.matmul(
                        outp[:], lhsT=wc[:], rhs=pooled_sb[i][:],
                        start=(i == 0), stop=(i == nchunk_built - 1),
                    )
                outsb = smpool.tile([O, P], f32, name="outsb", tag="outsb")
                nc.scalar.copy(outsb[:], outp[:])
                nc.sync.dma_start(out=out_e[:, :], in_=outsb[:])

    nc.compile()
    return nc


def make_in_maps(x, W1s, b1s, W2s, b2s, gammas, betas, Wout, cfg, hd):
    N, F, L, C = cfg["N"], cfg["F"], cfg["L"], cfg["C"]
    NP, T = hd["NP"], hd["T"]
    x = np.asarray(x, dtype=np.float32)
    x_table = x.astype(ml_dtypes.bfloat16)
    w1 = np.ascontiguousarray(np.asarray(W1s, np.float32))
    w2 = np.ascontiguousarray(np.asarray(W2s, np.float32))
    b1t = np.ascontiguousarray(np.asarray(b1s, np.float32).T)
    b2t = np.ascontiguousarray(np.asarray(b2s, np.float32).T)
    gamt = np.ascontiguousarray(np.asarray(gammas, np.float32).T)
    bett = np.ascontiguousarray(np.asarray(betas, np.float32).T)
    wout = np.ascontiguousarray(np.asarray(Wout, np.float32))
    idx_rep = np.zeros((C, P, hd["tot_rows"] // 16), dtype=np.int16)
    for g in range(P // 16):
        idx_rep[:, g * 16 : (g + 1) * 16, :] = hd["idx16"]
    in_maps = []
    for c in range(C):
        xo = np.zeros((T * P, F), dtype=ml_dtypes.bfloat16)
        xo[:NP] = x_table[c * NP : (c + 1) * NP]
        in_maps.append(
            dict(
                x_table=x_table, x_own=xo,
                idx16=np.ascontiguousarray(idx_rep[c]),
                dst_loc=np.ascontiguousarray(hd["dst_loc"][c]),
                batch_loc=np.ascontiguousarray(hd["batch_loc"][c]),
                iota_rep=hd["iota_rep"], ident=hd["ident"], identbf=hd["identbf"],
                W1s=w1, W2s=w2, b1T=b1t, b2T=b2t, gamT=gamt, betT=bett,
                Wout=wout,
            )
        )
    return in_maps


def assemble_output(results, bout, cfg, hd):
    G, O, C = cfg["G"], cfg["O"], cfg["C"]
    out = np.zeros((G, O), dtype=np.float64)
    for c in range(C):
        part = np.asarray(results[c]["outT"], dtype=np.float64)  # [O, P]
        g0 = int(hd["gbase"][c])
        n = min(P, G - g0)
        out[g0 : g0 + n] += part[:, :n].T
    out += np.asarray(bout, dtype=np.float64)
    return out.astype(np.float32)


def run(inputs, cfg=None, trace=False):
    cfg = cfg or FULL_CFG
    from concourse import bass_utils

    hd = prep_host_data(inputs["x"], inputs["edge_index"], inputs["batch"], cfg)
    eps_host = np.asarray(inputs["eps"], dtype=np.float64)
    nc = build_program(cfg, hd, eps_host)
    in_maps = make_in_maps(
        inputs["x"], inputs["W1s"], inputs["b1s"], inputs["W2s"], inputs["b2s"],
        inputs["gammas"], inputs["betas"], inputs["Wout"], cfg, hd,
    )
    res = bass_utils.run_bass_kernel_spmd(
        nc, in_maps, core_ids=list(range(cfg["C"])), trace=trace
    )
    out = assemble_output(res.results, inputs["bout"], cfg, hd)
    return out, res


def kernel(**inputs):
    out, _ = run(inputs, FULL_CFG, trace=False)
    return out
